# revision 1
# baseline (speedup 1.0000x reference)
"""Stage-3 Trainium2 Bass kernel for nn_BilinearFullSymLoss.

Per-sample math (derivation in kernel2.py / kernel_v1.py):
  delta(i,j) = wA0*G(i,j) + wA1*G(i+1,j) + wB0*bc(i,j) + wB1*bc(i+1,j)
  bc(i,j)    = cb0*G(i+rb, j+cb) + cb1*G(i+rb, j+cb+1)   (col interp)
  pos: wA=(1,0),         wB=(-(1-fy),-fy), rb=dy1,   cb=dx1
  neg: wA=(-fy,-(1-fy)), wB=(1,0),         rb=dy1+1, cb=dx1
       (neg evaluated at j' = j-dx1; host sums columns [-dx1, W))
  loss = m^2 * sum(valid delta^2) / (rows*cols); host does the scalar math.

Device plan per core (4 samples):
- combine channels (ACT scale-copy || DVE tensor_scalar, then DVE add)
  -> G fp16 in SBUF, written back to a per-sample DRAM scratch region
- ONE dynamic-offset window win[p,q,:] = Gd[off + i*W .. +W+2) via sync
  HWDGE DMA (off loaded from SBUF into an SP register; this walrus build
  rejects runtime-assert and multi-wait instructions, hence
  skip_runtime_bounds_check and the _split_multiwaits post-pass)
- bc = col-interp of win (DVE tensor_scalar 4x fp16 + tensor_tensor 2x)
- delta accumulated in PSUM by PE matmuls with host-built banded lhsT
  (wA0*I + wA1*subdiag) @ G + (wB0*I + wB1*subdiag) @ bc, plus
  single-entry cross-seam matrices for the row-128 boundaries
- ACT Square(PSUM) -> sq fp16; PE matmul with a 0/1 ivalid stationary
  vector gives i-masked per-column sums; host sums the valid column range
"""

import sys

sys.path.insert(0, "/opt/trn_rl_repo")

import numpy as np

import concourse.bass as bass
import concourse.tile as tile
from concourse import mybir
from concourse.bass_utils import run_bass_kernel_spmd

H = 512
W = 512
P = 128
Q = H // P
NS = 4
NCORES = 8
RPAD = 16
REG = (H + RPAD) * W
WLEN = W + 2

F32 = mybir.dt.float32
F16 = mybir.dt.float16
I32 = mybir.dt.int32

NPF = 4   # a, b, cb0, cb1
COL_A, COL_B, COL_CB0, COL_CB1 = range(NPF)

_CACHE = {}


def _split_multiwaits(nc):
    """The staged walrus accepts one sync wait per instruction; hoist extras
    onto single-wait NoOps."""
    n = 0
    for fn in nc.m.functions:
        for bb in fn.blocks:
            newlist = []
            for ins in bb.instructions:
                si = ins.sync_info
                if si is not None and si.on_wait is not None and len(si.on_wait) > 1:
                    waits = list(si.on_wait)
                    for w in waits[:-1]:
                        n += 1
                        newlist.append(mybir.InstNoOp(
                            name=f"WSPLIT-{n}-{ins.name}", opcode="NoOp",
                            engine=ins.engine,
                            sync_info=mybir.SyncInfo(on_wait=[w], on_update=[])))
                    ins.sync_info = mybir.SyncInfo(
                        on_wait=[waits[-1]], on_update=list(si.on_update))
                newlist.append(ins)
            bb.instructions = newlist
    return n


def _grid_ap(g, s, c):
    return g[s, c].rearrange("(q p) w -> p q w", p=P)


def _flat_ap(gd, offset, inner):
    return bass.AP(tensor=gd, offset=offset, ap=[[W, P], [P * W, Q], [1, inner]])


def _build_program():
    nc = bass.Bass("TRN2", target_bir_lowering=False, debug=False)

    g = nc.dram_tensor("g", [NS, 2, H, W], F32, kind="ExternalInput")
    pf = nc.dram_tensor("pf", [P, NS * NPF], F32, kind="ExternalInput")
    pi = nc.dram_tensor("pi", [1, 2 * NS], I32, kind="ExternalInput")
    iv = nc.dram_tensor("iv", [P, NS * Q], F16, kind="ExternalInput")
    mats = nc.dram_tensor("mats", [P, NS * 2 * P], F16, kind="ExternalInput")
    xmats = nc.dram_tensor("xmats", [P, NS * 2 * P], F16, kind="ExternalInput")
    out = nc.dram_tensor("out", [NS, W], F32, kind="ExternalOutput")
    RH = 272
    gdl = nc.dram_tensor("gdl", [NS * RH * W, 1], F16)
    gdh = nc.dram_tensor("gdh", [NS * RH * W, 1], F16)

    with tile.TileContext(nc) as tc:
        with (
            tc.tile_pool(name="consts", bufs=1) as consts,
            tc.tile_pool(name="io", bufs=2) as io,
            tc.tile_pool(name="work", bufs=2) as work,
            tc.tile_pool(name="psd", bufs=2, space="PSUM") as psdp,
        ):
            g0_first = io.tile([P, Q, W], F32, tag="g0", name="g0_0")
            nc.sync.dma_start(g0_first[:], _grid_ap(g, 0, 0))
            g1_first = io.tile([P, Q, W], F32, tag="g1", name="g1_0")
            nc.sync.dma_start(g1_first[:], _grid_ap(g, 0, 1))

            pfsb = consts.tile([P, NS * NPF], F32)
            nc.sync.dma_start(pfsb[:], pf[:])
            pisb = consts.tile([1, 2 * NS], I32)
            nc.sync.dma_start(pisb[:], pi[:])
            ivsb = consts.tile([P, NS * Q], F16)
            nc.sync.dma_start(ivsb[:], iv[:])
            matsb = consts.tile([P, NS * 2 * P], F16)
            nc.sync.dma_start(matsb[:], mats[:])
            xmatsb = consts.tile([P, NS * 2 * P], F16)
            nc.sync.dma_start(xmatsb[:], xmats[:])
            osb = consts.tile([1, NS * W], F32)

            zp = consts.tile([P, RPAD * W // P], F16)
            nc.vector.memset(zp[:], 0.0)
            for s in range(NS):
                nc.sync.dma_start(
                    bass.AP(tensor=gdh, offset=s * RH * W + 256 * W,
                            ap=[[RPAD * W // P, P], [1, RPAD * W // P]]),
                    zp[:],
                )

            for s in range(NS):
                pcol = lambda c: pfsb[:, s * NPF + c: s * NPF + c + 1]
                mA = matsb[:, (2 * s) * P:(2 * s + 1) * P]
                mB = matsb[:, (2 * s + 1) * P:(2 * s + 2) * P]
                xA = xmatsb[:, (2 * s) * P:(2 * s + 1) * P]
                xB = xmatsb[:, (2 * s + 1) * P:(2 * s + 2) * P]

                if s == 0:
                    g0sb, g1sb = g0_first, g1_first
                else:
                    g0sb = io.tile([P, Q, W], F32, tag="g0", name=f"g0_{s}")
                    nc.sync.dma_start(g0sb[:], _grid_ap(g, s, 0))
                    g1sb = io.tile([P, Q, W], F32, tag="g1", name=f"g1_{s}")
                    nc.sync.dma_start(g1sb[:], _grid_ap(g, s, 1))

                # G = a*g0 + b*g1 -> fp16, split lo/hi so the lower
                # writeback+window decouple from the upper combine; the
                # 16-row overlap is recomputed into its own tiny tile
                gsb = work.tile([P, Q, W], F16)
                gsbx = work.tile([16, 1, W], F16)
                for (lo_q, hi_q, tag) in ((0, 2, "lo"), (2, 4, "hi")):
                    th0 = work.tile([P, 2, W], F16, tag=f"t0{tag}",
                                    name=f"t0{tag}_{s}")
                    nc.scalar.activation(th0[:], g0sb[:, lo_q:hi_q, :],
                                         mybir.ActivationFunctionType.Copy,
                                         scale=pcol(COL_A))
                    th1 = work.tile([P, 2, W], F16, tag=f"t1{tag}",
                                    name=f"t1{tag}_{s}")
                    nc.vector.tensor_scalar(
                        out=th1[:], in0=g1sb[:, lo_q:hi_q, :],
                        scalar1=pcol(COL_B),
                        scalar2=None, op0=mybir.AluOpType.mult)
                    nc.vector.tensor_tensor(out=gsb[:, lo_q:hi_q, :],
                                            in0=th0[:], in1=th1[:],
                                            op=mybir.AluOpType.add)
                    if tag == "lo":
                        tx0 = work.tile([16, 1, W], F16, tag="tx0",
                                        name=f"tx0_{s}")
                        nc.scalar.activation(
                            tx0[:], g0sb[0:16, 2:3, :],
                            mybir.ActivationFunctionType.Copy,
                            scale=pfsb[0:16, s * NPF + COL_A:
                                       s * NPF + COL_A + 1])
                        tx1 = work.tile([16, 1, W], F16, tag="tx1",
                                        name=f"tx1_{s}")
                        nc.vector.tensor_scalar(
                            out=tx1[:], in0=g1sb[0:16, 2:3, :],
                            scalar1=pfsb[0:16, s * NPF + COL_B:
                                         s * NPF + COL_B + 1],
                            scalar2=None, op0=mybir.AluOpType.mult)
                        nc.vector.tensor_tensor(out=gsbx[:], in0=tx0[:],
                                                in1=tx1[:],
                                                op=mybir.AluOpType.add)
                        nc.sync.dma_start(
                            bass.AP(tensor=gdl, offset=s * RH * W,
                                    ap=[[W, P], [P * W, 2], [1, W]]),
                            gsb[:, 0:2, :])
                        nc.sync.dma_start(
                            bass.AP(tensor=gdl,
                                    offset=s * RH * W + 256 * W,
                                    ap=[[W, 16], [1, W]]),
                            gsbx[:])
                offl = nc.values_load(pisb[0:1, 2 * s: 2 * s + 1],
                                      engines=(mybir.EngineType.SP,),
                                      skip_runtime_bounds_check=True)
                winl = io.tile([P, 2, WLEN], F16)
                nc.sync.dma_start(
                    winl[:], bass.AP(tensor=gdl, offset=offl,
                                     ap=[[W, P], [P * W, 2], [1, WLEN]]))
                nc.sync.dma_start(
                    bass.AP(tensor=gdh, offset=s * RH * W,
                            ap=[[W, P], [P * W, 2], [1, W]]),
                    gsb[:, 2:4, :])
                offh = nc.values_load(pisb[0:1, 2 * s + 1: 2 * s + 2],
                                      engines=(mybir.EngineType.SP,),
                                      skip_runtime_bounds_check=True)
                winh = io.tile([P, 2, WLEN], F16)
                nc.sync.dma_start(
                    winh[:], bass.AP(tensor=gdh, offset=offh,
                                     ap=[[W, P], [P * W, 2], [1, WLEN]]))

                bc = work.tile([P, Q, W], F16)
                for hv, wsrc in ((0, winl), (1, winh)):
                    bch = work.tile([P, 2, W], F16, tag=f"bch{hv}",
                                    name=f"bch{hv}_{s}")
                    nc.vector.tensor_scalar(
                        out=bch[:], in0=wsrc[:, :, 0:W], scalar1=pcol(COL_CB0),
                        scalar2=None, op0=mybir.AluOpType.mult)
                    bc1 = work.tile([P, 2, W], F16, tag=f"bc1{hv}",
                                    name=f"bc1{hv}_{s}")
                    nc.vector.tensor_scalar(
                        out=bc1[:], in0=wsrc[:, :, 1:W + 1],
                        scalar1=pcol(COL_CB1),
                        scalar2=None, op0=mybir.AluOpType.mult)
                    nc.vector.tensor_tensor(out=bc[:, 2 * hv:2 * hv + 2, :],
                                            in0=bch[:], in1=bc1[:],
                                            op=mybir.AluOpType.add)

                # delta (per 128-row block) accumulated in PSUM via PE
                psd = psdp.tile([P, Q, W], F32)
                for q in range(Q):
                    mms = [(mA, gsb[:, q, :]), (mB, bc[:, q, :])]
                    if q < Q - 1:
                        mms += [(xA, gsb[:, q + 1, :]), (xB, bc[:, q + 1, :])]
                    for k, (lhsT, rhs) in enumerate(mms):
                        nc.tensor.matmul(psd[:, q, :], lhsT=lhsT, rhs=rhs,
                                         start=(k == 0), stop=(k == len(mms) - 1))

                # square -> fp16, then ivalid-weighted column sums on PE
                sq = work.tile([P, Q, W], F16)
                nc.scalar.activation(sq[:], psd[:],
                                     mybir.ActivationFunctionType.Square)
                ps = psd[0:1, 0, 0:W]
                for q in range(Q):
                    nc.tensor.matmul(
                        ps, lhsT=ivsb[:, s * Q + q: s * Q + q + 1],
                        rhs=sq[:, q, :], start=(q == 0), stop=(q == Q - 1))
                nc.vector.tensor_copy(osb[0:1, s * W:(s + 1) * W], ps)

            nc.sync.dma_start(out[:], osb[0:1, :])

    return nc


def _host_params(gt_sym_axis, gd_sym_axis):
    B = gt_sym_axis.shape[0]
    gt = gt_sym_axis.astype(np.float32)
    gds = gd_sym_axis.astype(np.float32)
    prm = []
    for i in range(B):
        sx = gds[i, 0]
        sy = gds[i, 1]
        dx = np.float32(-10.0) * gt[i, 0]
        dy = np.float32(10.0) * gt[i, 1]
        dy1f = np.float32(np.floor(dy))
        dx1f = np.float32(np.floor(dx))
        dy1 = int(dy1f)
        dx1 = int(dx1f)
        fy = np.float32(dy - dy1f)
        fx = np.float32(dx - dx1f)
        pos = bool(dx > 0)
        one = np.float32(1.0)
        zero = np.float32(0.0)
        if pos:
            wa = (one, zero)
            wb = (-(one - fy), -fy)
            rb, cb = dy1, dx1
            jlo, jhi = 0, W - dx1 - 1
        else:
            wa = (-fy, -(one - fy))
            wb = (one, zero)
            rb, cb = dy1 + 1, dx1
            jlo, jhi = -dx1, W
        rows = H - dy1 - 1
        cols = (W - dx1 - 1) if pos else (W + dx1)
        m = max(abs(float(sx)), abs(float(sy)), 1e-30)
        a = np.float32(float(sy) / m)
        b = np.float32(float(sx) / m)
        wf = np.array([a, b, one - fx, fx], dtype=np.float32)
        assert 0 <= rb <= RPAD - 4 and -16 <= cb <= 16 and 0 <= jlo <= jhi <= W
        prm.append(dict(wf=wf, wa=wa, wb=wb, rb=rb, cb=cb, jlo=jlo, jhi=jhi,
                        rows=rows, cols=cols, scale=m * m))
    return prm


def _band(w0, w1):
    """lhsT[k, m] = w0*d(k==m) + w1*d(k==m+1)."""
    mat = np.zeros((P, P), np.float16)
    idx = np.arange(P)
    mat[idx, idx] = np.float16(w0)
    mat[idx[1:], idx[:-1]] = np.float16(w1)
    return mat


def _xband(w1):
    """cross-seam lhsT[k, m] = w1*d(k==0, m==127)."""
    mat = np.zeros((P, P), np.float16)
    mat[0, P - 1] = np.float16(w1)
    return mat


def kernel(grid, gt_sym_axis, gd_sym_axis):
    grid = np.ascontiguousarray(grid, dtype=np.float32)
    B = grid.shape[0]
    assert grid.shape == (B, 2, H, W) and B == NS * NCORES

    if "nc" not in _CACHE:
        nc = _build_program()
        _split_multiwaits(nc)
        _CACHE["nc"] = nc
    nc = _CACHE["nc"]

    prm = _host_params(np.asarray(gt_sym_axis), np.asarray(gd_sym_axis))

    i_of_pq = np.arange(H).reshape(Q, P).T
    in_maps = []
    for c in range(NCORES):
        pfv = np.zeros((P, NS * NPF), np.float32)
        piv = np.zeros((1, 2 * NS), np.int32)
        ivv = np.zeros((P, NS * Q), np.float16)
        matv = np.zeros((P, NS * 2 * P), np.float16)
        xmatv = np.zeros((P, NS * 2 * P), np.float16)
        for s in range(NS):
            p = prm[c * NS + s]
            pfv[:, s * NPF:(s + 1) * NPF] = p["wf"][None, :]
            piv[0, 2 * s] = s * 272 * W + p["rb"] * W + p["cb"]
            piv[0, 2 * s + 1] = s * 272 * W + p["rb"] * W + p["cb"]
            ivv[:, s * Q:(s + 1) * Q] = (i_of_pq < p["rows"]).astype(np.float16)
            matv[:, (2 * s) * P:(2 * s + 1) * P] = _band(*p["wa"])
            matv[:, (2 * s + 1) * P:(2 * s + 2) * P] = _band(*p["wb"])
            xmatv[:, (2 * s) * P:(2 * s + 1) * P] = _xband(p["wa"][1])
            xmatv[:, (2 * s + 1) * P:(2 * s + 2) * P] = _xband(p["wb"][1])
        in_maps.append({
            "g": grid[c * NS:(c + 1) * NS],
            "pf": pfv, "pi": piv, "iv": ivv, "mats": matv, "xmats": xmatv,
        })

    res = run_bass_kernel_spmd(nc, in_maps, core_ids=list(range(NCORES)))

    losses = np.zeros(B, np.float64)
    for c in range(NCORES):
        o = res.results[c]["out"]
        for s in range(NS):
            p = prm[c * NS + s]
            ssq = float(o[s, p["jlo"]:p["jhi"]].sum(dtype=np.float64))
            count = float(np.float32(p["rows"] * p["cols"]))
            losses[c * NS + s] = p["scale"] * ssq / count
    return np.float32(losses.mean())



# revision 2
# speedup vs baseline: 1.0866x; 1.0866x over previous
"""Trainium2 Bass kernel for nn_BilinearFullSymLoss (v2).

Per-sample math (validated against reference in fp64):
  delta(i,j) = wa0*G(i,j) + wa1*G(i+1,j) + wb0*bc(i,j) + wb1*bc(i+1,j)
  bc(i,j)    = cb0*W(i,j) + cb1*W(i,j+1),  W(i,j) = G(i+rb, j+cb)
  pos: wa=(1,0),        wb=(-(1-fy),-fy), rb=dy1,   cb=dx1, valid j<[0,W-dx1-1)
  neg: wa=(-fy,-(1-fy)), wb=(1,0),        rb=dy1+1, cb=dx1, valid j in [-dx1,W)
  loss = m^2 * sum(valid delta^2) / (rows*cols)

Device plan per core (4 samples), all heavy traffic fp16:
- Pool SWDGE casting DMA loads g0,g1 (f32 DRAM -> f16 SBUF)
- DVE: G = a*g0 + b*g1  (ts, ts, tt; fp16 fast modes)
- G written once to a per-sample DRAM scratch (fp16); ONE dynamic-offset
  window read win[p,q,0:W+1] = Gd[rb*W+cb + r*W + j] handles both the row
  and the column shift (offset loaded from SBUF into an SP register)
- PE accumulates delta in PSUM with host-built banded lhsT:
    psd[:,q,:] = mA@G[:,q,:] + mB0@win[:,q,0:W] + mB1@win[:,q,1:W+1]
  mA = wa0*I + wa1*sub, mB0 = cb0*(wb0*I+wb1*sub), mB1 = cb1*(...), all with
  out-row 127 zeroed (cross-seam rows 127/255/383/511 are fixed exactly on
  the host from the f32 grid -- no cross-seam matmuls needed)
- ACT: sq = Square(psd) -> fp16
- PE: ivalid-weighted column sums (masks invalid rows; scratch tail is
  zero-padded so no NaNs enter), DVE copies [1,W] to SBUF, one DMA out.
Host: sums the valid column range, adds the seam rows, scales, means.
"""

import sys

sys.path.insert(0, "/opt/trn_rl_repo")

import numpy as np

import concourse.bass as bass
import concourse.tile as tile
from concourse import mybir
from concourse.bass_utils import run_bass_kernel_spmd

H = 512
W = 512
P = 128
Q = H // P
NS = 4
NCORES = 8
WLEN = W + 1
RPAD = 8
R = (H + RPAD) * W  # per-sample scratch elements

F32 = mybir.dt.float32
F16 = mybir.dt.float16
I32 = mybir.dt.int32

NPF = 2  # a, b
COL_A, COL_B = range(NPF)

_CACHE = {}


def _split_multiwaits(nc):
    """The staged walrus accepts one sync wait per instruction; hoist extras
    onto single-wait NoOps."""
    n = 0
    for fn in nc.m.functions:
        for bb in fn.blocks:
            newlist = []
            for ins in bb.instructions:
                si = ins.sync_info
                if si is not None and si.on_wait is not None and len(si.on_wait) > 1:
                    waits = list(si.on_wait)
                    for w in waits[:-1]:
                        n += 1
                        newlist.append(mybir.InstNoOp(
                            name=f"WSPLIT-{n}-{ins.name}", opcode="NoOp",
                            engine=ins.engine,
                            sync_info=mybir.SyncInfo(on_wait=[w], on_update=[])))
                    ins.sync_info = mybir.SyncInfo(
                        on_wait=[waits[-1]], on_update=list(si.on_update))
                newlist.append(ins)
            bb.instructions = newlist
    return n


def _grid_ap(g, s, c):
    return g[s, c].rearrange("(q p) w -> p q w", p=P)


def _build_program():
    nc = bass.Bass("TRN2", target_bir_lowering=False, debug=False)

    g = nc.dram_tensor("g", [NS, 2, H, W], F32, kind="ExternalInput")
    pf = nc.dram_tensor("pf", [P, NS * NPF], F32, kind="ExternalInput")
    pi = nc.dram_tensor("pi", [1, NS], I32, kind="ExternalInput")
    iv = nc.dram_tensor("iv", [P, NS * Q], F16, kind="ExternalInput")
    mats = nc.dram_tensor("mats", [P, NS * 3 * P], F16, kind="ExternalInput")
    out = nc.dram_tensor("out", [NS, W], F32, kind="ExternalOutput")
    gds = [nc.dram_tensor(f"gd{s}", [R, 1], F16) for s in range(NS)]

    with tile.TileContext(nc) as tc:
        with (
            tc.tile_pool(name="consts", bufs=1) as consts,
            tc.tile_pool(name="io", bufs=2) as io,
            tc.tile_pool(name="work", bufs=2) as work,
            tc.tile_pool(name="psd", bufs=2, space="PSUM") as psdp,
        ):
            # first sample's input loads go out before the consts
            gh0_first = io.tile([P, Q, W], F16, tag="gh0", name="gh0_0")
            nc.gpsimd.dma_start(gh0_first[:], _grid_ap(g, 0, 0))
            gh1_first = io.tile([P, Q, W], F16, tag="gh1", name="gh1_0")
            nc.gpsimd.dma_start(gh1_first[:], _grid_ap(g, 0, 1))

            pfsb = consts.tile([P, NS * NPF], F32)
            nc.sync.dma_start(pfsb[:], pf[:])
            pisb = consts.tile([1, NS], I32)
            nc.sync.dma_start(pisb[:], pi[:])
            ivsb = consts.tile([P, NS * Q], F16)
            nc.sync.dma_start(ivsb[:], iv[:])
            matsb = consts.tile([P, NS * 3 * P], F16)
            nc.sync.dma_start(matsb[:], mats[:])
            osb = consts.tile([1, NS * W], F32)

            # zero-pad the scratch tails (rows H..H+RPAD)
            zp = consts.tile([P, RPAD * W // P], F16)
            nc.vector.memset(zp[:], 0.0)
            for s in range(NS):
                nc.sync.dma_start(
                    bass.AP(tensor=gds[s], offset=H * W,
                            ap=[[RPAD * W // P, P], [1, RPAD * W // P]]),
                    zp[:])

            for s in range(NS):
                pcol = lambda c: pfsb[:, s * NPF + c: s * NPF + c + 1]
                mA = matsb[:, (3 * s) * P:(3 * s + 1) * P]
                mB0 = matsb[:, (3 * s + 1) * P:(3 * s + 2) * P]
                mB1 = matsb[:, (3 * s + 2) * P:(3 * s + 3) * P]

                if s == 0:
                    gh0, gh1 = gh0_first, gh1_first
                else:
                    gh0 = io.tile([P, Q, W], F16, tag="gh0", name=f"gh0_{s}")
                    nc.gpsimd.dma_start(gh0[:], _grid_ap(g, s, 0))
                    gh1 = io.tile([P, Q, W], F16, tag="gh1", name=f"gh1_{s}")
                    nc.gpsimd.dma_start(gh1[:], _grid_ap(g, s, 1))

                # G = a*g0 + b*g1 (DVE fp16 fast path)
                t0 = work.tile([P, Q, W], F16, tag="t0", name=f"t0_{s}")
                nc.vector.tensor_scalar(
                    out=t0[:], in0=gh0[:], scalar1=pcol(COL_A),
                    scalar2=None, op0=mybir.AluOpType.mult)
                t1 = work.tile([P, Q, W], F16, tag="t1", name=f"t1_{s}")
                nc.vector.tensor_scalar(
                    out=t1[:], in0=gh1[:], scalar1=pcol(COL_B),
                    scalar2=None, op0=mybir.AluOpType.mult)
                gsb = work.tile([P, Q, W], F16, tag="G", name=f"G_{s}")
                nc.vector.tensor_tensor(out=gsb[:], in0=t0[:], in1=t1[:],
                                        op=mybir.AluOpType.add)

                # scratch write + dynamic window read (row+col shift)
                nc.sync.dma_start(
                    bass.AP(tensor=gds[s], offset=0,
                            ap=[[W, P], [P * W, Q], [1, W]]),
                    gsb[:])
                off = nc.values_load(pisb[0:1, s: s + 1],
                                     engines=(mybir.EngineType.SP,),
                                     skip_runtime_bounds_check=True)
                win = io.tile([P, Q, WLEN], F16, tag="win", name=f"win_{s}")
                nc.sync.dma_start(
                    win[:], bass.AP(tensor=gds[s], offset=off,
                                    ap=[[W, P], [P * W, Q], [1, WLEN]]))

                # delta accumulated in PSUM by PE band matmuls
                psd = psdp.tile([P, Q, W], F32)
                for q in range(Q):
                    nc.tensor.matmul(psd[:, q, :], lhsT=mA, rhs=gsb[:, q, :],
                                     start=True, stop=False)
                    nc.tensor.matmul(psd[:, q, :], lhsT=mB0,
                                     rhs=win[:, q, 0:W],
                                     start=False, stop=False)
                    nc.tensor.matmul(psd[:, q, :], lhsT=mB1,
                                     rhs=win[:, q, 1:WLEN],
                                     start=False, stop=True)

                # square -> fp16, ivalid-weighted column sums on PE
                sq = work.tile([P, Q, W], F16, tag="sq", name=f"sq_{s}")
                nc.scalar.activation(sq[:], psd[:],
                                     mybir.ActivationFunctionType.Square)
                ps = psd[0:1, 0, 0:W]
                for q in range(Q):
                    nc.tensor.matmul(
                        ps, lhsT=ivsb[:, s * Q + q: s * Q + q + 1],
                        rhs=sq[:, q, :], start=(q == 0), stop=(q == Q - 1))
                nc.vector.tensor_copy(osb[0:1, s * W:(s + 1) * W], ps)

            nc.sync.dma_start(out[:], osb[0:1, :])

    return nc


def _host_params(gt_sym_axis, gd_sym_axis):
    B = gt_sym_axis.shape[0]
    gt = gt_sym_axis.astype(np.float32)
    gds = gd_sym_axis.astype(np.float32)
    prm = []
    for i in range(B):
        sx = gds[i, 0]
        sy = gds[i, 1]
        dx = np.float32(-10.0) * gt[i, 0]
        dy = np.float32(10.0) * gt[i, 1]
        dy1f = np.float32(np.floor(dy))
        dx1f = np.float32(np.floor(dx))
        dy1 = int(dy1f)
        dx1 = int(dx1f)
        fy = np.float32(dy - dy1f)
        fx = np.float32(dx - dx1f)
        pos = bool(dx > 0)
        one = np.float32(1.0)
        zero = np.float32(0.0)
        if pos:
            wa = (one, zero)
            wb = (-(one - fy), -fy)
            rb, cb = dy1, dx1
            jlo, jhi = 0, W - dx1 - 1
        else:
            wa = (-fy, -(one - fy))
            wb = (one, zero)
            rb, cb = dy1 + 1, dx1
            jlo, jhi = -dx1, W
        rows = H - dy1 - 1
        cols = (W - dx1 - 1) if pos else (W + dx1)
        m = max(abs(float(sx)), abs(float(sy)), 1e-30)
        a = np.float32(float(sy) / m)
        b = np.float32(float(sx) / m)
        assert 1 <= rb * W + cb and rb * W + cb + H * W + W <= R + W
        assert 0 <= rb <= RPAD - 2 and -W // 2 <= cb <= W // 2
        assert 0 <= jlo <= jhi <= W
        prm.append(dict(a=a, b=b, wa=wa, wb=wb, rb=rb, cb=cb,
                        cb0=one - fx, cb1=fx, jlo=jlo, jhi=jhi,
                        rows=rows, cols=cols, scale=m * m))
    return prm


def _band(w0, w1):
    """lhsT[k, m] = w0*d(k==m) + w1*d(k==m+1), out-row 127 zeroed."""
    mat = np.zeros((P, P), np.float16)
    idx = np.arange(P)
    mat[idx, idx] = np.float16(w0)
    mat[idx[1:], idx[:-1]] = np.float16(w1)
    mat[:, P - 1] = np.float16(0.0)
    return mat


def _seam_fix(grid_s, p):
    """Exact fp64 contribution of the device-zeroed rows 127/255/383/511."""
    g0 = grid_s[0].astype(np.float64)
    g1 = grid_s[1].astype(np.float64)
    G = p["a"] * g0 + p["b"] * g1
    Gp = np.vstack([G, np.zeros((RPAD, W))])
    flat = Gp.reshape(-1)
    wa0, wa1 = float(p["wa"][0]), float(p["wa"][1])
    wb0, wb1 = float(p["wb"][0]), float(p["wb"][1])
    cb0, cb1 = float(p["cb0"]), float(p["cb1"])
    base = p["rb"] * W + p["cb"]
    jlo, jhi = p["jlo"], p["jhi"]
    ssq = 0.0
    for r in (127, 255, 383, 511):
        if r >= p["rows"]:
            continue
        w_r = flat[base + r * W: base + r * W + W + 1]
        w_r1 = flat[base + (r + 1) * W: base + (r + 1) * W + W + 1]
        bc_r = cb0 * w_r[0:W] + cb1 * w_r[1:W + 1]
        bc_r1 = cb0 * w_r1[0:W] + cb1 * w_r1[1:W + 1]
        g_r1 = G[r + 1] if r + 1 < H else np.zeros(W)
        d = wa0 * G[r] + wa1 * g_r1 + wb0 * bc_r + wb1 * bc_r1
        ssq += float((d[jlo:jhi] ** 2).sum())
    return ssq


def kernel(grid, gt_sym_axis, gd_sym_axis):
    grid = np.ascontiguousarray(grid, dtype=np.float32)
    B = grid.shape[0]
    assert grid.shape == (B, 2, H, W) and B == NS * NCORES

    if "nc" not in _CACHE:
        nc = _build_program()
        _split_multiwaits(nc)
        _CACHE["nc"] = nc
    nc = _CACHE["nc"]

    prm = _host_params(np.asarray(gt_sym_axis), np.asarray(gd_sym_axis))

    i_of_pq = np.arange(H).reshape(Q, P).T
    in_maps = []
    for c in range(NCORES):
        pfv = np.zeros((P, NS * NPF), np.float32)
        piv = np.zeros((1, NS), np.int32)
        ivv = np.zeros((P, NS * Q), np.float16)
        matv = np.zeros((P, NS * 3 * P), np.float16)
        for s in range(NS):
            p = prm[c * NS + s]
            pfv[:, s * NPF + COL_A] = p["a"]
            pfv[:, s * NPF + COL_B] = p["b"]
            piv[0, s] = p["rb"] * W + p["cb"]
            ivv[:, s * Q:(s + 1) * Q] = (i_of_pq < p["rows"]).astype(np.float16)
            matv[:, (3 * s) * P:(3 * s + 1) * P] = _band(*p["wa"])
            bb = _band(*p["wb"])
            matv[:, (3 * s + 1) * P:(3 * s + 2) * P] = (
                bb * np.float16(p["cb0"]))
            matv[:, (3 * s + 2) * P:(3 * s + 3) * P] = (
                bb * np.float16(p["cb1"]))
        in_maps.append({
            "g": grid[c * NS:(c + 1) * NS],
            "pf": pfv, "pi": piv, "iv": ivv, "mats": matv,
        })

    res = run_bass_kernel_spmd(nc, in_maps, core_ids=list(range(NCORES)))

    losses = np.zeros(B, np.float64)
    for c in range(NCORES):
        o = res.results[c]["out"]
        for s in range(NS):
            p = prm[c * NS + s]
            ssq = float(o[s, p["jlo"]:p["jhi"]].sum(dtype=np.float64))
            ssq += _seam_fix(grid[c * NS + s], p)
            count = float(np.float32(p["rows"] * p["cols"]))
            losses[c * NS + s] = p["scale"] * ssq / count
    return np.float32(losses.mean())


# revision 7
# speedup vs baseline: 1.1885x; 1.0937x over previous
"""Trainium2 Bass kernel for nn_BilinearFullSymLoss (v3).

Per-sample math (validated against reference in fp64):
  delta(i,j) = wa0*G(i,j) + wa1*G(i+1,j) + wb0*bc(i,j) + wb1*bc(i+1,j)
  bc(i,j)    = cb0*Wn(i,j) + cb1*Wn(i,j+1),  Wn(i,j) = G(i+rb, j+cb)
  pos: wa=(1,0),         wb=(-(1-fy),-fy), rb=dy1,   cb=dx1, valid j in [0,W-dx1-1)
  neg: wa=(-fy,-(1-fy)), wb=(1,0),         rb=dy1+1, cb=dx1, valid j in [-dx1,W)
  loss = m^2 * sum(valid delta^2) / (rows*cols)

Device plan per core (4 samples), all heavy traffic fp16:
- ONE Pool SWDGE casting DMA per sample loads both channels (f32->f16)
- DVE: G = a*g0 + b*g1 per half (ts, ts, tt; fp16 fast modes); the scratch
  writes are issued from the DVE queue right after each half is ready
- G goes to per-sample lower/upper DRAM scratch tensors (lower carries an
  8-row overlap strip; upper carries a zeroed 8-row tail) and comes back via
  TWO dynamic-offset window reads win[p,q,0:W+1] = Gd[rb*W+cb + r*W + j]
  (row+column shift in one offset, loaded into an SP register)
- PE accumulates delta in PSUM with host-built banded lhsT:
    psd[:,q,:] = mA@G[:,q,:] + mB0@win[:,q,0:W] + mB1@win[:,q,1:W+1]
  mA = wa0*I + wa1*sub, mB0 = cb0*(wb0*I + wb1*sub), mB1 = cb1*(same band),
  all with out-row 127 zeroed: seam rows 127/255/383/511 are added back
  exactly on the host from the f32 grid, so no cross-seam matmuls exist
- ACT: sq = Square(psd) per half -> fp16
- PE: ivalid-weighted column sums (invalid q=3 rows masked; zero-padded
  scratch keeps NaNs out); colsum for sample s is emitted after sample
  s+1's delta matmuls so PE never stalls on ACT
- DVE copies [1,W] to SBUF; per-sample DMA writes the output row.
Host: sums the valid column range, adds seam rows, scales, means.
"""

import sys

sys.path.insert(0, "/opt/trn_rl_repo")

import numpy as np

import concourse.bass as bass
import concourse.tile as tile
from concourse import mybir
from concourse.bass_utils import run_bass_kernel_spmd

H = 512
W = 512
P = 128
Q = H // P
NS = 4
NCORES = 8
WLEN = W + 1
RPAD = 8
RL = (2 * P + RPAD) * W   # lower scratch: rows 0..255 + 8-row overlap strip
RH = (2 * P + RPAD) * W   # upper scratch: rows 256..511 + zeroed tail

F32 = mybir.dt.float32
F16 = mybir.dt.float16
I32 = mybir.dt.int32

NPF = 2  # a, b
COL_A, COL_B = range(NPF)

_CACHE = {}


def _split_multiwaits(nc):
    """The staged walrus accepts one sync wait per instruction; hoist extras
    onto single-wait NoOps."""
    n = 0
    for fn in nc.m.functions:
        for bb in fn.blocks:
            newlist = []
            for ins in bb.instructions:
                si = ins.sync_info
                if si is not None and si.on_wait is not None and len(si.on_wait) > 1:
                    waits = list(si.on_wait)
                    for w in waits[:-1]:
                        n += 1
                        newlist.append(mybir.InstNoOp(
                            name=f"WSPLIT-{n}-{ins.name}", opcode="NoOp",
                            engine=ins.engine,
                            sync_info=mybir.SyncInfo(on_wait=[w], on_update=[])))
                    ins.sync_info = mybir.SyncInfo(
                        on_wait=[waits[-1]], on_update=list(si.on_update))
                newlist.append(ins)
            bb.instructions = newlist
    return n


def _build_program():
    nc = bass.Bass("TRN2", target_bir_lowering=False, debug=False)

    g = nc.dram_tensor("g", [NS, 2, H, W], F32, kind="ExternalInput")
    pf = nc.dram_tensor("pf", [P, NS * NPF], F32, kind="ExternalInput")
    pi = nc.dram_tensor("pi", [1, NS], I32, kind="ExternalInput")
    iv = nc.dram_tensor("iv", [P, NS * Q], F16, kind="ExternalInput")
    mats = nc.dram_tensor("mats", [P, NS * 3 * P], F16, kind="ExternalInput")
    out = nc.dram_tensor("out", [NS, W], F32, kind="ExternalOutput")
    gdl = [nc.dram_tensor(f"gdl{s}", [RL, 1], F16) for s in range(NS)]
    gdh = [nc.dram_tensor(f"gdh{s}", [RH, 1], F16) for s in range(NS)]

    with tile.TileContext(nc) as tc:
        with (
            tc.tile_pool(name="consts", bufs=1) as consts,
            tc.tile_pool(name="io", bufs=3) as io,
            tc.tile_pool(name="work", bufs=2) as work,
            tc.tile_pool(name="psd", bufs=2, space="PSUM") as psdp,
        ):
            # sample-0 input load first in line for the DMA engines
            ghs = []
            gh = io.tile([P, 2 * Q, W], F16, tag="gh", name="gh_0")
            nc.gpsimd.dma_start(
                gh[:],
                bass.AP(tensor=g, offset=0,
                        ap=[[W, P], [H * W, 2], [P * W, Q], [1, W]]))
            ghs.append(gh)

            pfsb = consts.tile([P, NS * NPF], F32)
            nc.sync.dma_start(pfsb[:], pf[:])
            pisb = consts.tile([1, NS], I32)
            nc.sync.dma_start(pisb[:], pi[:])
            ivsb = consts.tile([P, NS * Q], F16)
            nc.sync.dma_start(ivsb[:], iv[:])
            matsb = consts.tile([P, NS * 3 * P], F16)
            nc.sync.dma_start(matsb[:], mats[:])
            osb = consts.tile([1, NS * W], F32)

            # zero the upper-scratch tails (window overreach past row 511)
            zp = consts.tile([P, RPAD * W // P], F16)
            nc.vector.memset(zp[:], 0.0)
            for s in range(NS):
                nc.sync.dma_start(
                    bass.AP(tensor=gdh[s], offset=2 * P * W,
                            ap=[[RPAD * W // P, P], [1, RPAD * W // P]]),
                    zp[:])

            # remaining input loads (Pool queue; all ready at t=0)
            for s in range(1, NS):
                gh = io.tile([P, 2 * Q, W], F16, tag="gh", name=f"gh_{s}")
                nc.gpsimd.dma_start(
                    gh[:],
                    bass.AP(tensor=g, offset=s * 2 * H * W,
                            ap=[[W, P], [H * W, 2], [P * W, Q], [1, W]]))
                ghs.append(gh)

            stage = []  # per-sample state for the software pipeline

            def emit_front(s):
                """combine -> scratch writes -> window reads -> delta mms."""
                pcol = lambda c: pfsb[:, s * NPF + c: s * NPF + c + 1]
                mA = matsb[:, (3 * s) * P:(3 * s + 1) * P]
                mB0 = matsb[:, (3 * s + 1) * P:(3 * s + 2) * P]
                mB1 = matsb[:, (3 * s + 2) * P:(3 * s + 3) * P]
                gh = ghs[s]

                gsb = work.tile([P, Q, W], F16, tag="G", name=f"G_{s}")
                off = nc.values_load(pisb[0:1, s: s + 1],
                                     engines=(mybir.EngineType.SP,),
                                     skip_runtime_bounds_check=True)
                wins = []
                for half, (lo, hi) in enumerate(((0, 2), (2, 4))):
                    t0 = work.tile([P, 2, W], F16, tag=f"t0{half}",
                                   name=f"t0{half}_{s}")
                    nc.vector.tensor_scalar(
                        out=t0[:], in0=gh[:, lo:hi, :], scalar1=pcol(COL_A),
                        scalar2=None, op0=mybir.AluOpType.mult)
                    t1 = work.tile([P, 2, W], F16, tag=f"t1{half}",
                                   name=f"t1{half}_{s}")
                    nc.vector.tensor_scalar(
                        out=t1[:], in0=gh[:, Q + lo:Q + hi, :],
                        scalar1=pcol(COL_B),
                        scalar2=None, op0=mybir.AluOpType.mult)
                    nc.vector.tensor_tensor(out=gsb[:, lo:hi, :], in0=t0[:],
                                            in1=t1[:], op=mybir.AluOpType.add)
                    gd = gdl[s] if half == 0 else gdh[s]
                    nc.sync.dma_start(
                        bass.AP(tensor=gd, offset=0,
                                ap=[[W, P], [P * W, 2], [1, W]]),
                        gsb[:, lo:hi, :])
                    if half == 0:
                        # recompute the 8-row overlap strip (rows 256..263)
                        # so the lower window read need not wait for the
                        # upper-half combine
                        tx0 = work.tile([RPAD, 1, W], F16, tag="tx0",
                                        name=f"tx0_{s}")
                        nc.vector.tensor_scalar(
                            out=tx0[:], in0=gh[0:RPAD, 2:3, :],
                            scalar1=pfsb[0:RPAD,
                                         s * NPF + COL_A: s * NPF + COL_A + 1],
                            scalar2=None, op0=mybir.AluOpType.mult)
                        tx1 = work.tile([RPAD, 1, W], F16, tag="tx1",
                                        name=f"tx1_{s}")
                        nc.vector.tensor_scalar(
                            out=tx1[:], in0=gh[0:RPAD, Q + 2:Q + 3, :],
                            scalar1=pfsb[0:RPAD,
                                         s * NPF + COL_B: s * NPF + COL_B + 1],
                            scalar2=None, op0=mybir.AluOpType.mult)
                        txs = work.tile([RPAD, 1, W], F16, tag="txs",
                                        name=f"txs_{s}")
                        nc.vector.tensor_tensor(out=txs[:], in0=tx0[:],
                                                in1=tx1[:],
                                                op=mybir.AluOpType.add)
                        nc.sync.dma_start(
                            bass.AP(tensor=gdl[s], offset=2 * P * W,
                                    ap=[[W, RPAD], [1, W]]),
                            txs[:])
                    # window read for this half (SP queue, dynamic offset)
                    win = io.tile([P, 2, WLEN], F16, tag=f"win{half}",
                                  name=f"win{half}_{s}")
                    nc.sync.dma_start(
                        win[:], bass.AP(tensor=gd, offset=off,
                                        ap=[[W, P], [P * W, 2], [1, WLEN]]))
                    wins.append(win)

                # the overlap strip must land before the lower window read:
                # both touch gdl[s]; tile's DRAM dep tracking orders them.
                psd = psdp.tile([P, Q, W], F32)
                sq = work.tile([P, Q, W], F16, tag="sq", name=f"sq_{s}")
                for half, (lo, hi) in enumerate(((0, 2), (2, 4))):
                    win = wins[half]
                    for qq in range(2):
                        q = lo + qq
                        nc.tensor.matmul(psd[:, q, :], lhsT=mA,
                                         rhs=gsb[:, q, :],
                                         start=True, stop=False)
                        nc.tensor.matmul(psd[:, q, :], lhsT=mB0,
                                         rhs=win[:, qq, 0:W],
                                         start=False, stop=False)
                        nc.tensor.matmul(psd[:, q, :], lhsT=mB1,
                                         rhs=win[:, qq, 1:WLEN],
                                         start=False, stop=True)
                    # square this half while PE moves on
                    nc.scalar.activation(sq[:, lo:hi, :], psd[:, lo:hi, :],
                                         mybir.ActivationFunctionType.Square)
                return dict(psd=psd, sq=sq)

            def emit_back(s):
                """ivalid-weighted column sums -> SBUF -> per-sample out."""
                st = stage[s]
                ps = st["psd"][0:1, 0, 0:W]
                for q in range(Q):
                    nc.tensor.matmul(
                        ps, lhsT=ivsb[:, s * Q + q: s * Q + q + 1],
                        rhs=st["sq"][:, q, :], start=(q == 0), stop=(q == Q - 1))
                nc.vector.tensor_copy(osb[0:1, s * W:(s + 1) * W], ps)
                nc.sync.dma_start(out[s: s + 1, :], osb[0:1, s * W:(s + 1) * W])

            # software pipeline: colsum of sample s-1 after sample s's mms
            for s in range(NS):
                stage.append(emit_front(s))
                if s > 0:
                    emit_back(s - 1)
            emit_back(NS - 1)

    return nc


def _host_params(gt_sym_axis, gd_sym_axis):
    B = gt_sym_axis.shape[0]
    gt = gt_sym_axis.astype(np.float32)
    gds = gd_sym_axis.astype(np.float32)
    prm = []
    for i in range(B):
        sx = gds[i, 0]
        sy = gds[i, 1]
        dx = np.float32(-10.0) * gt[i, 0]
        dy = np.float32(10.0) * gt[i, 1]
        dy1f = np.float32(np.floor(dy))
        dx1f = np.float32(np.floor(dx))
        dy1 = int(dy1f)
        dx1 = int(dx1f)
        fy = np.float32(dy - dy1f)
        fx = np.float32(dx - dx1f)
        pos = bool(dx > 0)
        one = np.float32(1.0)
        zero = np.float32(0.0)
        if pos:
            wa = (one, zero)
            wb = (-(one - fy), -fy)
            rb, cb = dy1, dx1
            jlo, jhi = 0, W - dx1 - 1
        else:
            wa = (-fy, -(one - fy))
            wb = (one, zero)
            rb, cb = dy1 + 1, dx1
            jlo, jhi = -dx1, W
        rows = H - dy1 - 1
        cols = (W - dx1 - 1) if pos else (W + dx1)
        m = max(abs(float(sx)), abs(float(sy)), 1e-30)
        a = np.float32(float(sy) / m)
        b = np.float32(float(sx) / m)
        off = rb * W + cb
        assert 1 <= off and off + (2 * P - 1) * W + WLEN <= RL
        assert 0 <= rb <= RPAD - 2 and -16 <= cb <= 16
        assert 0 <= jlo <= jhi <= W
        prm.append(dict(a=a, b=b, wa=wa, wb=wb, rb=rb, cb=cb,
                        cb0=one - fx, cb1=fx, jlo=jlo, jhi=jhi,
                        rows=rows, cols=cols, scale=m * m))
    return prm


def _band(w0, w1):
    """lhsT[k, m] = w0*d(k==m) + w1*d(k==m+1), out-row 127 zeroed."""
    mat = np.zeros((P, P), np.float16)
    idx = np.arange(P)
    mat[idx, idx] = np.float16(w0)
    mat[idx[1:], idx[:-1]] = np.float16(w1)
    mat[:, P - 1] = np.float16(0.0)
    return mat


def _seam_fix(grid_s, p):
    """Exact fp64 contribution of the device-zeroed rows 127/255/383/511."""
    g0 = grid_s[0].astype(np.float64)
    g1 = grid_s[1].astype(np.float64)
    G = p["a"] * g0 + p["b"] * g1
    Gp = np.vstack([G, np.zeros((RPAD, W))])
    flat = Gp.reshape(-1)
    wa0, wa1 = float(p["wa"][0]), float(p["wa"][1])
    wb0, wb1 = float(p["wb"][0]), float(p["wb"][1])
    cb0, cb1 = float(p["cb0"]), float(p["cb1"])
    base = p["rb"] * W + p["cb"]
    jlo, jhi = p["jlo"], p["jhi"]
    ssq = 0.0
    for r in (127, 255, 383, 511):
        if r >= p["rows"]:
            continue
        w_r = flat[base + r * W: base + r * W + W + 1]
        w_r1 = flat[base + (r + 1) * W: base + (r + 1) * W + W + 1]
        bc_r = cb0 * w_r[0:W] + cb1 * w_r[1:W + 1]
        bc_r1 = cb0 * w_r1[0:W] + cb1 * w_r1[1:W + 1]
        g_r1 = G[r + 1] if r + 1 < H else np.zeros(W)
        d = wa0 * G[r] + wa1 * g_r1 + wb0 * bc_r + wb1 * bc_r1
        ssq += float((d[jlo:jhi] ** 2).sum())
    return ssq


def kernel(grid, gt_sym_axis, gd_sym_axis):
    grid = np.ascontiguousarray(grid, dtype=np.float32)
    B = grid.shape[0]
    assert grid.shape == (B, 2, H, W) and B == NS * NCORES

    if "nc" not in _CACHE:
        nc = _build_program()
        _split_multiwaits(nc)
        _CACHE["nc"] = nc
    nc = _CACHE["nc"]

    prm = _host_params(np.asarray(gt_sym_axis), np.asarray(gd_sym_axis))

    i_of_pq = np.arange(H).reshape(Q, P).T
    in_maps = []
    for c in range(NCORES):
        pfv = np.zeros((P, NS * NPF), np.float32)
        piv = np.zeros((1, NS), np.int32)
        ivv = np.zeros((P, NS * Q), np.float16)
        matv = np.zeros((P, NS * 3 * P), np.float16)
        for s in range(NS):
            p = prm[c * NS + s]
            pfv[:, s * NPF + COL_A] = p["a"]
            pfv[:, s * NPF + COL_B] = p["b"]
            piv[0, s] = p["rb"] * W + p["cb"]
            ivv[:, s * Q:(s + 1) * Q] = (i_of_pq < p["rows"]).astype(np.float16)
            matv[:, (3 * s) * P:(3 * s + 1) * P] = _band(*p["wa"])
            bb = _band(*p["wb"])
            matv[:, (3 * s + 1) * P:(3 * s + 2) * P] = (
                bb * np.float16(p["cb0"]))
            matv[:, (3 * s + 2) * P:(3 * s + 3) * P] = (
                bb * np.float16(p["cb1"]))
        in_maps.append({
            "g": grid[c * NS:(c + 1) * NS],
            "pf": pfv, "pi": piv, "iv": ivv, "mats": matv,
        })

    res = run_bass_kernel_spmd(nc, in_maps, core_ids=list(range(NCORES)))

    losses = np.zeros(B, np.float64)
    for c in range(NCORES):
        o = res.results[c]["out"]
        for s in range(NS):
            p = prm[c * NS + s]
            ssq = float(o[s, p["jlo"]:p["jhi"]].sum(dtype=np.float64))
            ssq += _seam_fix(grid[c * NS + s], p)
            count = float(np.float32(p["rows"] * p["cols"]))
            losses[c * NS + s] = p["scale"] * ssq / count
    return np.float32(losses.mean())
